# revision 13
# baseline (speedup 1.0000x reference)
"""Trainium2 Bass kernel for nn_Biaffine_57475252355702.

Model (B=8, P=64, S=512, D=1024, L=64, FF=2 with the "only last layer
survives" FFNN bug):
    pred = gather(span, predicates)                      [B,P,D]
    p_sc = relu(pred @ Wp1.T + bp1) @ w_p                [B,P]
    a_sc = relu(span @ Wa1.T + ba1) @ w_a                [B,S]
    first[b,p,s,l] = pred[b] @ W1[l] @ span[b].T
    combine = first + pred@W2p + span@W2a + bias + p_sc + a_sc
    combine[..., L-1] = 0
    return combine.reshape(B*P*S, L), labels.reshape(-1)

Sharding: labels are sharded 8-per-core (W1 slice is 32MB fp32 -> 16MB
bf16 per core); every core computes its 8 labels for ALL batches.  The
unary scores p_sc/a_sc (+ the rank-1 "rowterm" = pred@W2p + bias + p_sc)
are computed data-parallel (core c does batch c) and shared through a
tiny AllGather.  All rank-1 terms are folded into matmuls:
  - span@W2a is folded into tmpT during the stage-A PSUM->SBUF copy
    (per-partition scalar add of W2a[e,l]),
  - rowterm and a_sc are applied with one K=2 matmul appended to the
    stage-B accumulation group.
The null-label zeroing is a data-driven per-partition mask multiply
(only core 7's last pair masks), keeping the program SPMD-uniform.

Per-core output is [B, 4, 128, S] (pair-majored label blocks); the host
reassembles [B*P*S, L].
"""
import numpy as np
import ml_dtypes
from contextlib import ExitStack

B, P, S, D, L = 8, 64, 512, 1024, 64
NCORES = 8
LLOC = L // NCORES          # 8 labels per core
NPAIR = LLOC // 2           # 4 label pairs per core
NDT = D // 128              # 8 tiles over the contraction dims
bf16 = ml_dtypes.bfloat16

_CACHE: dict = {}


def _build_program():
    import concourse.bacc as bacc
    import concourse.tile as tile
    import concourse.mybir as mybir

    dt = mybir.dt
    BF = dt.bfloat16
    F32 = dt.float32
    Relu = mybir.ActivationFunctionType.Relu
    ADD = mybir.AluOpType.add
    MULT = mybir.AluOpType.mult

    nc = bacc.Bacc("TRN2", target_bir_lowering=False, debug=False,
                   num_devices=NCORES)

    def inp(name, shape, dtype=BF):
        return nc.dram_tensor(name, shape, dtype, kind="ExternalInput").ap()

    spanT = inp("spanT", [B, NDT, 128, S])            # [b, et, pp, s]
    spanOwnT = inp("spanOwnT", [NDT, 128, S])         # own batch [dt, pp, s]
    predT = inp("predT", [NDT, 128, B * P])           # [dt, pp, (b,p)]
    predOwnT = inp("predOwnT", [NDT, 128, P])         # own batch [dt, pp, p]
    W1t = inp("W1t", [LLOC, NDT, 128, NDT, 128])      # [l, et, pp, dt, e]
    Wa1T = inp("Wa1T", [NDT, 128, NDT, 128])          # [jt, pp, dt, j]
    Wp1T = inp("Wp1T", [NDT, 128, NDT, 128])
    waT = inp("waT", [128, NDT])                      # [pp, jt]
    wpT = inp("wpT", [128, NDT])
    ba1T = inp("ba1T", [128, NDT], F32)
    bp1T = inp("bp1T", [128, NDT], F32)
    W2pT = inp("W2pT", [NDT, 128, L])                 # [dt, pp, l]
    W2aLoc = inp("W2aLoc", [NDT, 128, LLOC], F32)     # [et, pp, lloc]
    bbRow = inp("bbRow", [1, L])                      # bf16 bias row
    onesD = inp("onesD", [1, S])                      # bf16 row of ones
    rtsel = inp("rtsel", [L, LLOC])                   # one-hot label selector
    maskT = inp("maskT", [128, NPAIR], F32)           # null-label mask
    out = nc.dram_tensor("out", [B, NPAIR, 128, S], F32,
                         kind="ExternalOutput").ap()

    UW = S + L * P  # 512 + 4096: a_score row + rowterm [l, p] flat

    with tile.TileContext(nc) as tc, ExitStack() as ctx:
        const = ctx.enter_context(tc.tile_pool(name="const", bufs=1))
        wstream = ctx.enter_context(tc.tile_pool(name="wstream", bufs=4))
        hidp = ctx.enter_context(tc.tile_pool(name="hidp", bufs=3))
        outp = ctx.enter_context(tc.tile_pool(name="outp", bufs=3))
        smallp = ctx.enter_context(tc.tile_pool(name="smallp", bufs=2))
        psA = ctx.enter_context(tc.tile_pool(name="psA", bufs=2, space="PSUM"))
        psB = ctx.enter_context(tc.tile_pool(name="psB", bufs=2, space="PSUM"))
        psU = ctx.enter_context(tc.tile_pool(name="psU", bufs=2, space="PSUM"))
        psV = ctx.enter_context(tc.tile_pool(name="psV", bufs=2, space="PSUM"))
        dramp = ctx.enter_context(tc.tile_pool(name="dramp", bufs=1, space="DRAM"))

        # ---------- resident small loads ----------
        predT_sb = []
        for dti in range(NDT):
            t = const.tile([128, B * P], BF, tag=f"predT{dti}")
            nc.sync.dma_start(t[:], predT[dti])
            predT_sb.append(t)
        spanOwn_sb = []
        predOwn_sb = []
        for dti in range(NDT):
            t = const.tile([128, S], BF, tag=f"spanOwn{dti}")
            nc.sync.dma_start(t[:], spanOwnT[dti])
            spanOwn_sb.append(t)
            t = const.tile([128, P], BF, tag=f"predOwn{dti}")
            nc.sync.dma_start(t[:], predOwnT[dti])
            predOwn_sb.append(t)
        W2pT_sb = []
        W2a_sb = []
        for dti in range(NDT):
            t = const.tile([128, L], BF, tag=f"W2pT{dti}")
            nc.sync.dma_start(t[:], W2pT[dti])
            W2pT_sb.append(t)
            t = const.tile([128, LLOC], F32, tag=f"W2a{dti}")
            nc.sync.dma_start(t[:], W2aLoc[dti])
            W2a_sb.append(t)
        waT_sb = const.tile([128, NDT], BF, tag="waT")
        nc.sync.dma_start(waT_sb[:], waT[:])
        wpT_sb = const.tile([128, NDT], BF, tag="wpT")
        nc.sync.dma_start(wpT_sb[:], wpT[:])
        ba1T_sb = const.tile([128, NDT], F32, tag="ba1T")
        nc.sync.dma_start(ba1T_sb[:], ba1T[:])
        bp1T_sb = const.tile([128, NDT], F32, tag="bp1T")
        nc.sync.dma_start(bp1T_sb[:], bp1T[:])
        bbRow_sb = const.tile([1, L], BF, tag="bbRow")
        nc.sync.dma_start(bbRow_sb[:], bbRow[:])
        maskT_sb = const.tile([128, NPAIR], F32, tag="maskT")
        nc.sync.dma_start(maskT_sb[:], maskT[:])
        rtsel_sb = const.tile([L, LLOC], BF, tag="rtsel")
        nc.sync.dma_start(rtsel_sb[:], rtsel[:])
        ones64 = const.tile([1, L], BF, tag="ones64")
        nc.vector.memset(ones64[:], 1.0)

        # ---------- phase 1: unary FFNN scores (own batch only) ----------
        def ffnn_score(WT_ap, rhs_sb, bias_sb, wT_sb, width, tagp):
            ps_score = psV.tile([1, width], F32, tag="psV")
            for jt in range(NDT):
                wtile = wstream.tile([128, NDT, 128], BF, tag="wstream")
                nc.sync.dma_start(wtile[:], WT_ap[jt])
                ps_h = psU.tile([128, width], F32, tag="psU")
                for dti in range(NDT):
                    nc.tensor.matmul(ps_h[:], wtile[:, dti, :], rhs_sb[dti][:],
                                     start=(dti == 0), stop=(dti == NDT - 1))
                hid = hidp.tile([128, width], BF, tag="hidp")
                nc.scalar.activation(hid[:], ps_h[:], Relu,
                                     bias=bias_sb[:, jt:jt + 1])
                nc.tensor.matmul(ps_score[:], wT_sb[:, jt:jt + 1], hid[:],
                                 start=(jt == 0), stop=(jt == NDT - 1))
            sc = smallp.tile([1, width], BF, tag=f"score{tagp}")
            nc.vector.tensor_copy(sc[:], ps_score[:])
            return sc

        asc_sb = ffnn_score(Wa1T, spanOwn_sb, ba1T_sb, waT_sb, S, "a")
        psc_sb = ffnn_score(Wp1T, predOwn_sb, bp1T_sb, wpT_sb, P, "p")

        # ---------- phase 2: rowterm[l, p] = (pred@W2p).T + bias[l] + p_sc[p]
        ps_rt = psV.tile([L, P], F32, tag="psV")
        for dti in range(NDT):
            nc.tensor.matmul(ps_rt[:], W2pT_sb[dti][:], predOwn_sb[dti][:],
                             start=(dti == 0), stop=False)
        # + bias[l] (x) ones[p]
        nc.tensor.matmul(ps_rt[:], bbRow_sb[:], ones64[:, 0:P],
                         start=False, stop=False)
        # + ones[l] (x) p_sc[p]
        nc.tensor.matmul(ps_rt[:], ones64[:], psc_sb[:],
                         start=False, stop=True)
        rt_sb = smallp.tile([L, P], BF, tag="rt")
        nc.vector.tensor_copy(rt_sb[:], ps_rt[:])

        # ---------- phase 3: AllGather of [a_score | rowterm] ----------
        uown = dramp.tile([1, UW], BF, tag="uown")
        uall = dramp.tile([B, UW], BF, tag="uall", addr_space="Shared")
        nc.sync.dma_start(uown[0:1, 0:S], asc_sb[:])
        nc.sync.dma_start(uown[0:1, S:UW], rt_sb[:])
        nc.gpsimd.collective_compute(
            "AllGather",
            mybir.AluOpType.bypass,
            replica_groups=[list(range(NCORES))],
            ins=[uown.opt()],
            outs=[uall.opt()],
        )

        # ---------- phase 4 (stage A): tmpT[l][e, (b,2,p)] = W1[l].T-contract
        # tmpT[(lp,et)][pp, b, lsub, p] = sum_d W1[l,d,e] pred[b,p,d] + W2a[e,l]
        tmpT_sb = {}
        spanT_sb = {}
        for lp in range(NPAIR):
            for lsub in range(2):
                lloc = 2 * lp + lsub
                for et in range(NDT):
                    w1c = wstream.tile([128, NDT, 128], BF, tag="wstream")
                    nc.sync.dma_start(w1c[:], W1t[lloc, et])
                    ps_t = psA.tile([128, B * P], F32, tag="psA")
                    for dti in range(NDT):
                        nc.tensor.matmul(ps_t[:], w1c[:, dti, :],
                                         predT_sb[dti][:],
                                         start=(dti == 0), stop=(dti == NDT - 1))
                    if lsub == 0:
                        tmpT_sb[(lp, et)] = const.tile(
                            [128, B, 2, P], BF, tag=f"tmpT{lp}_{et}",
                            name=f"tmpT{lp}_{et}")
                    dst = tmpT_sb[(lp, et)][:, :, lsub, :]
                    nc.vector.tensor_scalar(
                        dst, ps_t[:], W2a_sb[et][:, lloc:lloc + 1], None, ADD)
            # interleave the big spanT loads (needed from stage B onward);
            # one batched DMA per b (et lands on the free dim)
            for b in (2 * lp, 2 * lp + 1):
                t = const.tile([128, NDT, S], BF, tag=f"spanT{b}",
                               name=f"spanT{b}")
                nc.sync.dma_start(
                    t[:], spanT[b].rearrange("et pp s -> pp et s"))
                spanT_sb[b] = t

        # ---------- phase 5 (stage B): out[r, s] per (b, pair) ----------
        # per-batch: pick this core's 8 labels out of the gathered 64-label
        # rowterm block with a one-hot matmul (rtsel is a per-core INPUT, so
        # the program stays SPMD-uniform; a direct slice offset would not be)
        lhsT2_b = {}
        rhs2_b = {}
        for b in range(B):
            rt_all = smallp.tile([L, P], BF, tag="rt_all")
            nc.sync.dma_start(rt_all[:], uall[b:b + 1, S:S + L * P])
            ps_sel = psV.tile([LLOC, P], F32, tag="psV")
            nc.tensor.matmul(ps_sel[:], rtsel_sb[:], rt_all[:],
                             start=True, stop=True)
            rt_loc = smallp.tile([LLOC, P], BF, tag="rt_loc")
            nc.vector.tensor_copy(rt_loc[:], ps_sel[:])
            # pack all 4 pairs' rank-1 operands once per b:
            # lhsT2_b row0 = ones, row1 = rowterm flat [1, 512] (lp-major)
            lt = const.tile([2, NPAIR * 128], BF, tag=f"lhsT2_{b}",
                            name=f"lhsT2_{b}")
            nc.vector.memset(lt[0:1, :], 1.0)
            nc.sync.dma_start(lt[1:2, :], rt_loc[:])
            lhsT2_b[b] = lt
            rh = const.tile([2, S], BF, tag=f"rhs2_{b}", name=f"rhs2_{b}")
            nc.sync.dma_start(rh[0:1, :], uall[b:b + 1, 0:S])
            nc.sync.dma_start(rh[1:2, :], onesD[:])
            rhs2_b[b] = rh
        for lp in range(NPAIR):
            for b in range(B):
                ps_o = psB.tile([128, S], F32, tag="psB")
                for et in range(NDT):
                    nc.tensor.matmul(ps_o[:], tmpT_sb[(lp, et)][:, b, :, :],
                                     spanT_sb[b][:, et, :],
                                     start=(et == 0), stop=False)
                nc.tensor.matmul(ps_o[:],
                                 lhsT2_b[b][:, lp * 128:(lp + 1) * 128],
                                 rhs2_b[b][:], start=False, stop=True)
                o_sb = outp.tile([128, S], F32, tag="outp")
                nc.vector.tensor_scalar(o_sb[:], ps_o[:],
                                        maskT_sb[:, lp:lp + 1], None, MULT)
                nc.sync.dma_start(out[b, lp], o_sb[:])

    nc.compile()
    return nc


def _get_program():
    if "nc" not in _CACHE:
        _CACHE["nc"] = _build_program()
    return _CACHE["nc"]


def _prep_shared(span, pred_repr, Wa1, Wp1, ba1, bp1, w_a, w_p, W2, bb):
    """Inputs identical on every core."""
    m = {}
    m["spanT"] = np.ascontiguousarray(span.transpose(0, 2, 1)).astype(bf16) \
        .reshape(B, NDT, 128, S)
    m["predT"] = np.ascontiguousarray(pred_repr.reshape(B * P, D).T) \
        .astype(bf16).reshape(NDT, 128, B * P)

    def wt_tiles(W):  # W[out j, in d] -> [jt, pp(d), dt, j]
        return np.ascontiguousarray(
            W.T.reshape(NDT, 128, NDT, 128).transpose(2, 1, 0, 3)
        ).astype(bf16)

    m["Wa1T"] = wt_tiles(Wa1)
    m["Wp1T"] = wt_tiles(Wp1)
    m["waT"] = np.ascontiguousarray(w_a.reshape(NDT, 128).T).astype(bf16)
    m["wpT"] = np.ascontiguousarray(w_p.reshape(NDT, 128).T).astype(bf16)
    m["ba1T"] = np.ascontiguousarray(ba1.reshape(NDT, 128).T).astype(np.float32)
    m["bp1T"] = np.ascontiguousarray(bp1.reshape(NDT, 128).T).astype(np.float32)
    m["W2pT"] = np.ascontiguousarray(W2[:D]).astype(bf16).reshape(NDT, 128, L)
    m["bbRow"] = np.ascontiguousarray(bb.reshape(1, L)).astype(bf16)
    m["onesD"] = np.ones((1, S), bf16)
    return m


def _make_in_maps(shared, span, pred_repr, W1, W2):
    in_maps = []
    for c in range(NCORES):
        m = dict(shared)
        lab0 = c * LLOC
        # W1 slice pre-tiled: [l, et, pp(d), dt, e]
        m["W1t"] = np.ascontiguousarray(
            W1[lab0:lab0 + LLOC]
            .reshape(LLOC, NDT, 128, NDT, 128)     # [l, dt, pp, et, e]
            .transpose(0, 3, 2, 1, 4)              # [l, et, pp, dt, e]
        ).astype(bf16)
        m["W2aLoc"] = np.ascontiguousarray(W2[D:, lab0:lab0 + LLOC]) \
            .astype(np.float32).reshape(NDT, 128, LLOC)
        m["spanOwnT"] = np.ascontiguousarray(span[c].T).astype(bf16) \
            .reshape(NDT, 128, S)
        m["predOwnT"] = np.ascontiguousarray(pred_repr[c].T).astype(bf16) \
            .reshape(NDT, 128, P)
        mask = np.ones((128, NPAIR), np.float32)
        if lab0 + LLOC == L:
            mask[64:128, NPAIR - 1] = 0.0  # null label (L-1) zeroing
        m["maskT"] = mask
        sel = np.zeros((L, LLOC), np.float32)
        sel[np.arange(lab0, lab0 + LLOC), np.arange(LLOC)] = 1.0
        m["rtsel"] = sel.astype(bf16)
        in_maps.append(m)
    return in_maps


def _assemble(results):
    full = np.empty((B, P, S, L), np.float32)
    for c in range(NCORES):
        o = results[c]["out"].reshape(B, NPAIR, 2, P, S)
        full[:, :, :, c * LLOC:(c + 1) * LLOC] = \
            o.transpose(0, 3, 4, 1, 2).reshape(B, P, S, LLOC)
    return full.reshape(B * P * S, L)


def kernel(**inputs):
    from concourse.bass_utils import run_bass_kernel_spmd

    span = np.asarray(inputs["span_repr"], dtype=np.float32)
    preds = np.asarray(inputs["predicates"], dtype=np.int32)
    labels = np.asarray(inputs["labels"], dtype=np.int32)
    Wp_lin = np.asarray(inputs["Wp_lin"], dtype=np.float32)
    bp_lin = np.asarray(inputs["bp_lin"], dtype=np.float32)
    Wa_lin = np.asarray(inputs["Wa_lin"], dtype=np.float32)
    ba_lin = np.asarray(inputs["ba_lin"], dtype=np.float32)
    w_p = np.asarray(inputs["w_p"], dtype=np.float32)
    w_a = np.asarray(inputs["w_a"], dtype=np.float32)
    W1 = np.asarray(inputs["W1"], dtype=np.float32)
    W2 = np.asarray(inputs["W2"], dtype=np.float32)
    bb = np.asarray(inputs["b"], dtype=np.float32)[0]

    pred_repr = np.take_along_axis(span, preds[..., None], axis=1)  # [B,P,D]

    nc = _get_program()
    shared = _prep_shared(span, pred_repr, Wa_lin[-1], Wp_lin[-1],
                          ba_lin[-1], bp_lin[-1], w_a, w_p, W2, bb)
    in_maps = _make_in_maps(shared, span, pred_repr, W1, W2)

    res = run_bass_kernel_spmd(nc, in_maps, list(range(NCORES)))

    scores = _assemble(res.results)
    real_labels = labels.reshape(-1).astype(np.int32)
    return scores, real_labels
